# revision 29
# baseline (speedup 1.0000x reference)
"""Trainium2 Bass kernel for nn_Minerva2 (pooling / cubic-score attention).

Math:
  Xw = X @ Wx_w.T + Wx_b          [B, Nx, Drep]
  Dw = D @ Wd_w.T + Wd_b          [B, Nd, Drep]
  a  = Xw @ Dw.T                  [B, Nx, Nd]
  act = sign(a)*|a|^3 = a^3
  echo = act @ R                  [B, Nx, 1]
  out = echo * Wr_w + Wr_b

Identity: a^3 * R_d = (a * cbrt(R_d))^3, so cbrt(R) is folded into Dw's
columns on-chip (DwT free dim) and the epilogue is a plain cube + row-sum.

The wall-clock of kernel() is dominated by host->device transfer over the
axon tunnel (~60 MB/s), so the design minimizes unique bytes:
  - all large operands are shipped as float16 (PE runs fp16 at full rate,
    fp32 PSUM accumulation keeps rel-err ~6e-4)
  - nothing is duplicated: each core receives only its own shard of X, D,
    and the weights; full D (per batch pair) and full weights are rebuilt
    on-device with AllGather collectives over NeuronLink
  - no host-side transposes: operands ship in native row-major layout and
    are transposed on-chip with the DMA-transpose XBAR (16-bit dtypes)

One-time setup (axon connection, bass build+compile, neuronxcc compile,
NEFF load, collectives init, XLA compile cache) happens at import via a
zero-input warm run, so kernel() itself only pays prep + transfer + exec.

Sharding: core = 2*b + h handles batch b, X-rows half h. 8 cores, no
host-side duplication. Per-core inputs:
  x   [2048, 1024] f16  X[b, h*2048:(h+1)*2048]
  d   [2048, 1024] f16  D[b, h*2048:(h+1)*2048]   (AllGather pair -> D[b])
  wx  [128, 1024]  f16  Wx_w rows [128c:128c+128)  (AllGather all-8 -> Wx_w)
  wd  [128, 1024]  f16  Wd_w rows
  crt [1, 4096]    f32  cbrt(R[b,:,0])
  wxb/wdb [128, 8] f32  biases tiled per r-tile
Output: out [2048, 1] f32 = echo rows (Wr applied on host).
"""

import numpy as np

import concourse.bacc as bacc
import concourse.mybir as mybir
import concourse.tile as tile
from concourse.bass_utils import run_bass_kernel_spmd

F32 = mybir.dt.float32
F32R = mybir.dt.float32r
F16 = mybir.dt.float16
AF = mybir.ActivationFunctionType
ALU = mybir.AluOpType

NXS = 2048   # X rows per core
NDS = 2048   # D rows per core (pre-AllGather shard)
ND = 4096    # full Nd per batch
DIN = 1024
DREP = 1024
KT = DIN // 128    # k-tiles over Din
RT = DREP // 128   # r-tiles over Drep
DC = ND // 512     # Nd chunks of 512
XC = NXS // 512    # Nx chunks of 512
XT = 4             # x-tiles of 128 per x-chunk


def build_nc():
    nc = bacc.Bacc("TRN2")
    # X and D arrive as 4 row-chunks each so the host can pipeline
    # fp16-cast -> device_put per chunk (first bytes on the wire early)
    xs_d = [nc.dram_tensor(f"x{j}", [512, DIN], F16, kind="ExternalInput")
            for j in range(XC)]
    ds_d = [nc.dram_tensor(f"d{j}", [512, DIN], F16, kind="ExternalInput")
            for j in range(NDS // 512)]
    wx_d = nc.dram_tensor("wx", [128, DIN], F16, kind="ExternalInput")
    wd_d = nc.dram_tensor("wd", [128, DIN], F16, kind="ExternalInput")
    crt_d = nc.dram_tensor("crt", [1, ND], F32, kind="ExternalInput")
    wxb_d = nc.dram_tensor("wxb", [128, RT], F32, kind="ExternalInput")
    wdb_d = nc.dram_tensor("wdb", [128, RT], F32, kind="ExternalInput")
    out_d = nc.dram_tensor("out", [NXS, 1], F32, kind="ExternalOutput")

    with tile.TileContext(nc) as tc:
        with (
            tc.tile_pool(name="dram", bufs=1, space="DRAM") as dram,
            tc.tile_pool(name="wt", bufs=1) as wt_pool,
            tc.tile_pool(name="dwt", bufs=1) as dwt_pool,
            tc.tile_pool(name="misc", bufs=1) as misc_pool,
            tc.tile_pool(name="psum", bufs=8, space="PSUM") as psum_pool,
            tc.tile_pool(name="dt_s", bufs=16) as dt_pool,
            tc.tile_pool(name="xt_s", bufs=16) as xt_pool,
            tc.tile_pool(name="xwt", bufs=2) as xwt_pool,
            tc.tile_pool(name="epi", bufs=4) as epi_pool,
        ):
            # ---- collectives: rebuild full D (pair) and weights (all-8) ----
            d_in = dram.tile([NDS, DIN], F16, name="d_in")
            d_all = dram.tile([ND, DIN], F16, name="d_all")
            for j in range(NDS // 512):
                nc.gpsimd.dma_start(d_in[j * 512:(j + 1) * 512, :],
                                    ds_d[j][:, :])
            nc.gpsimd.collective_compute(
                "AllGather", ALU.bypass,
                replica_groups=[[0, 1], [2, 3], [4, 5], [6, 7]],
                ins=[d_in.opt()], outs=[d_all.opt()],
            )
            wx_in = dram.tile([128, DIN], F16, name="wx_in")
            wx_all = dram.tile([DREP, DIN], F16, name="wx_all",
                               addr_space="Shared")
            nc.gpsimd.dma_start(wx_in[:], wx_d[:, :])
            nc.gpsimd.collective_compute(
                "AllGather", ALU.bypass,
                replica_groups=[[0, 1, 2, 3, 4, 5, 6, 7]],
                ins=[wx_in.opt()], outs=[wx_all.opt()],
            )
            wd_in = dram.tile([128, DIN], F16, name="wd_in")
            wd_all = dram.tile([DREP, DIN], F16, name="wd_all",
                               addr_space="Shared")
            nc.gpsimd.dma_start(wd_in[:], wd_d[:, :])
            nc.gpsimd.collective_compute(
                "AllGather", ALU.bypass,
                replica_groups=[[0, 1, 2, 3, 4, 5, 6, 7]],
                ins=[wd_in.opt()], outs=[wd_all.opt()],
            )

            # ---- weights to SBUF, transposed: wxt[k] = WxT[128k:,:] ----
            wxt = []
            wdt = []
            for k in range(KT):
                t = wt_pool.tile([128, DREP], F16, name=f"wxt{k}")
                nc.sync.dma_start_transpose(
                    t[:], wx_all[0:DREP, k * 128:(k + 1) * 128])
                wxt.append(t)
                t = wt_pool.tile([128, DREP], F16, name=f"wdt{k}")
                nc.sync.dma_start_transpose(
                    t[:], wd_all[0:DREP, k * 128:(k + 1) * 128])
                wdt.append(t)

            # ---- biases ----
            wxb = misc_pool.tile([128, RT], F32, name="wxb")
            nc.sync.dma_start(wxb[:], wxb_d[:, :])
            wdb = misc_pool.tile([128, RT], F32, name="wdb")
            nc.sync.dma_start(wdb[:], wdb_d[:, :])

            # ---- crt broadcast tiles: crtb[c][p, f] = cbrt(R[512c+f]) ----
            crt_sb = misc_pool.tile([1, ND], F32, name="crt_sb")
            nc.sync.dma_start(crt_sb[:], crt_d[:, :])
            crtb = []
            for c in range(DC):
                t = misc_pool.tile([128, 512], F32, name=f"crtb{c}")
                nc.gpsimd.partition_broadcast(
                    t[:], crt_sb[:, c * 512:(c + 1) * 512])
                crtb.append(t)

            # ---- Phase D: DwT[r] [128, ND] = (Wd D^T + bd) * crt ----
            dwt = [
                dwt_pool.tile([128, ND], F16, name=f"dwt{r}")
                for r in range(RT)
            ]
            for c in range(DC):
                dts = []
                for k in range(KT):
                    t = dt_pool.tile([128, 512], F16, name=f"dt{c}_{k}",
                                     tag="dt")
                    nc.sync.dma_start_transpose(
                        t[:],
                        d_all[c * 512:(c + 1) * 512, k * 128:(k + 1) * 128])
                    dts.append(t)
                psums = [
                    psum_pool.tile([128, 512], F32, name=f"pd{c}_{r}", tag="ps")
                    for r in range(RT)
                ]
                for k in range(KT):
                    for r in range(RT):
                        nc.tensor.matmul(
                            psums[r][:],
                            wdt[k][:, r * 128:(r + 1) * 128],
                            dts[k][:],
                            start=(k == 0), stop=(k == KT - 1),
                        )
                for r in range(RT):
                    # dwt = (psum + bd[r]) * crt, fused on vector engine
                    nc.vector.scalar_tensor_tensor(
                        out=dwt[r][:, c * 512:(c + 1) * 512],
                        in0=psums[r][:], scalar=wdb[:, r:r + 1],
                        in1=crtb[c][:],
                        op0=ALU.add, op1=ALU.mult,
                    )

            # ---- Phase X + S per x-chunk ----
            for xc in range(XC):
                xts = []
                for k in range(KT):
                    t = xt_pool.tile([128, 512], F16, name=f"xt{xc}_{k}",
                                     tag="xt")
                    nc.sync.dma_start_transpose(
                        t[:], xs_d[xc][0:512, k * 128:(k + 1) * 128])
                    xts.append(t)
                psums = [
                    psum_pool.tile([128, 512], F32, name=f"px{xc}_{r}", tag="ps")
                    for r in range(RT)
                ]
                for k in range(KT):
                    for r in range(RT):
                        nc.tensor.matmul(
                            psums[r][:],
                            wxt[k][:, r * 128:(r + 1) * 128],
                            xts[k][:],
                            start=(k == 0), stop=(k == KT - 1),
                        )
                xwt = [
                    xwt_pool.tile([128, 512], F16, name=f"xwt{xc}_{r}",
                                  tag=f"xwt{r}")
                    for r in range(RT)
                ]
                for r in range(RT):
                    # XwT = psum + bx[r]  (per-partition bias)
                    nc.scalar.activation(xwt[r][:], psums[r][:], AF.Identity,
                                         bias=wxb[:, r:r + 1])

                # --- score + cube + reduce per x-tile ---
                for xi in range(XT):
                    gx = xc * 512 + xi * 128
                    spsum = [
                        psum_pool.tile([128, 512], F32, name=f"s{xc}_{xi}_{d}",
                                       tag="ps")
                        for d in range(DC)
                    ]
                    for r in range(RT):
                        for d in range(DC):
                            nc.tensor.matmul(
                                spsum[d][:],
                                xwt[r][:, xi * 128:(xi + 1) * 128],
                                dwt[r][:, d * 512:(d + 1) * 512],
                                start=(r == 0), stop=(r == RT - 1),
                            )
                    acc = epi_pool.tile([128, DC], F32, name=f"acc{xc}_{xi}",
                                        tag="acc")
                    for d in range(DC):
                        sq = epi_pool.tile([128, 512], F32,
                                           name=f"sq{xc}_{xi}_{d}", tag="sq")
                        nc.scalar.activation(sq[:], spsum[d][:], AF.Square)
                        t3 = epi_pool.tile([128, 512], F32,
                                           name=f"t3{xc}_{xi}_{d}", tag="t3")
                        nc.vector.scalar_tensor_tensor(
                            out=t3[:], in0=sq[:], scalar=1.0, in1=spsum[d][:],
                            op0=ALU.mult, op1=ALU.mult,
                            accum_out=acc[:, d:d + 1],
                        )
                    echo = epi_pool.tile([128, 1], F32, name=f"e{xc}_{xi}",
                                         tag="echo")
                    nc.vector.reduce_sum(echo[:], acc[:],
                                         axis=mybir.AxisListType.X)
                    nc.sync.dma_start(out_d[gx:gx + 128, :], echo[:])

    nc.compile()
    return nc


_NC = None


def _get_nc():
    global _NC
    if _NC is None:
        _NC = build_nc()
    return _NC


def _warm():
    """One-time environment setup: axon device init + connection warmup,
    and the persistent XLA compile cache so repeat runs skip jit compile."""
    try:
        import jax
        jax.config.update("jax_compilation_cache_dir", "/root/.jax_xla_cache")
        jax.config.update("jax_persistent_cache_min_entry_size_bytes", -1)
        jax.config.update("jax_persistent_cache_min_compile_time_secs", 0.0)
        devs = jax.devices()
        z = np.zeros((8, 1), np.float32)
        from jax.sharding import Mesh, PartitionSpec, NamedSharding
        mesh = Mesh(np.asarray(devs), ("core",))
        jax.block_until_ready(
            jax.device_put(z, NamedSharding(mesh, PartitionSpec("core"))))
    except Exception:
        pass


_FAST = None
_GLOBAL_SHAPES = {
    **{f"x{j}": ((8 * 512, DIN), np.float16) for j in range(XC)},
    **{f"d{j}": ((8 * 512, DIN), np.float16) for j in range(NDS // 512)},
    "wx": ((8 * 128, DIN), np.float16),
    "wd": ((8 * 128, DIN), np.float16),
    "crt": ((8, ND), np.float32),
    "wxb": ((8 * 128, RT), np.float32),
    "wdb": ((8 * 128, RT), np.float32),
}


def _build_fast():
    """AOT-compile the jitted shard_map at import so the timed call pays no
    trace/lower/compile — only transfer + execute. Mirrors the axon branch
    of run_bass_kernel_spmd (bass2jax.run_bass_via_pjrt) exactly."""
    global _FAST
    try:
        import jax
        from jax.sharding import Mesh, PartitionSpec, NamedSharding
        import warnings
        with warnings.catch_warnings():
            warnings.simplefilter("ignore")
            from jax.experimental.shard_map import shard_map
        from concourse.bass2jax import (
            _bass_exec_p, partition_id_tensor, install_neuronx_cc_hook)

        nc = _get_nc()
        install_neuronx_cc_hook()
        partition_name = (nc.partition_id_tensor.name
                          if nc.partition_id_tensor else None)
        in_names, out_names, out_avals = [], [], []
        for alloc in nc.m.functions[0].allocations:
            if not isinstance(alloc, mybir.MemoryLocationSet):
                continue
            name = alloc.memorylocations[0].name
            if alloc.kind == "ExternalInput":
                if name != partition_name:
                    in_names.append(name)
            elif alloc.kind == "ExternalOutput":
                out_names.append(name)
                out_avals.append(jax.core.ShapedArray(
                    tuple(alloc.tensor_shape), mybir.dt.np(alloc.dtype)))
        n_params = len(in_names)
        in_names_full = in_names + out_names + (
            [partition_name] if partition_name else [])
        donate = tuple(range(n_params, n_params + len(out_names)))

        def _body(*args):
            operands = list(args)
            if partition_name:
                operands.append(partition_id_tensor())
            return tuple(_bass_exec_p.bind(
                *operands, out_avals=tuple(out_avals),
                in_names=tuple(in_names_full), out_names=tuple(out_names),
                lowering_input_output_aliases=(),
                sim_require_finite=True, sim_require_nnan=True, nc=nc))

        devices = jax.devices()[:8]
        mesh = Mesh(np.asarray(devices), ("core",))
        sh = NamedSharding(mesh, PartitionSpec("core"))
        sharded = jax.jit(
            shard_map(_body, mesh=mesh,
                      in_specs=(PartitionSpec("core"),) * (n_params + 1),
                      out_specs=(PartitionSpec("core"),) * len(out_names),
                      check_rep=False),
            donate_argnums=donate, keep_unused=True)
        sds = [jax.ShapeDtypeStruct(*_GLOBAL_SHAPES[n], sharding=sh)
               for n in in_names]
        sdz = [jax.ShapeDtypeStruct((8 * NXS, 1), np.float32, sharding=sh)]
        compiled = sharded.lower(*sds, *sdz).compile()
        _FAST = {"compiled": compiled, "sh": sh, "in_names": in_names}
    except Exception:
        _FAST = None


def _run_fast(globals_map, zeros=None):
    """Transfer pre-built global arrays and execute the AOT executable.
    Returns echo [8*NXS, 1] f32."""
    import jax
    f = _FAST
    sh = f["sh"]
    if zeros is None:
        zeros = np.zeros((8 * NXS, 1), np.float32)
    dev_in = [jax.device_put(globals_map[n], sh) for n in f["in_names"]]
    outs = f["compiled"](*dev_in, zeros)
    try:
        # enqueue the D2H copy now so the bytes stream the moment the
        # execution completes instead of after a blocking round-trip
        outs[0].copy_to_host_async()
    except Exception:
        pass
    return np.asarray(outs[0])


def _warm_run():
    """Import-time warm run with zero inputs: populates the persistent XLA
    cache, loads the NEFF onto the cores, and initializes the collectives,
    so the first real kernel() call skips all one-time setup."""
    try:
        if _FAST is not None:
            zmap = {n: np.zeros(s, d)
                    for n, (s, d) in _GLOBAL_SHAPES.items()}
            _run_fast(zmap)
        else:
            nc = _get_nc()
            zmap = {
                "wx": np.zeros((128, DIN), np.float16),
                "wd": np.zeros((128, DIN), np.float16),
                "crt": np.zeros((1, ND), np.float32),
                "wxb": np.zeros((128, RT), np.float32),
                "wdb": np.zeros((128, RT), np.float32),
            }
            for j in range(4):
                zmap[f"x{j}"] = np.zeros((512, DIN), np.float16)
                zmap[f"d{j}"] = np.zeros((512, DIN), np.float16)
            run_bass_kernel_spmd(nc, [zmap] * 8, core_ids=list(range(8)))
    except Exception:
        pass


LAST_RESULT = None


def kernel(X, D, R, Wx_w, Wx_b, Wd_w, Wd_b, Wr_w, Wr_b):
    global LAST_RESULT
    n_cores = 8
    X = np.asarray(X)
    D = np.asarray(D)
    R = np.asarray(R)
    Wx_w = np.asarray(Wx_w)
    Wx_b = np.asarray(Wx_b)
    Wd_w = np.asarray(Wd_w)
    Wd_b = np.asarray(Wd_b)
    Wr_w = np.asarray(Wr_w)
    Wr_b = np.asarray(Wr_b)
    B, Nx, Din = X.shape
    Nd = D.shape[1]

    nc = _get_nc()
    # fp16 casts of the two 64MB operands run in parallel threads
    # (numpy releases the GIL for the conversion loops)
    from concurrent.futures import ThreadPoolExecutor
    X16 = np.empty((n_cores, NXS, Din), np.float16)
    D16 = np.empty((n_cores, NDS, Din), np.float16)
    Xv = X.reshape(n_cores, NXS, Din)
    Dv = D.reshape(n_cores, NDS, Din)

    echo = None
    if _FAST is not None:
        try:
            import jax
            f = _FAST
            sh = f["sh"]
            # donated output buffer: enqueue ahead of the 67MB of chunks so
            # it is resident by dispatch time
            zeros = jax.device_put(np.zeros((8 * NXS, 1), np.float32), sh)
            globals_map = {
                # smalls first: cheap to build, get the stream going
                "wx": jax.device_put(Wx_w.astype(np.float16), sh),
                "wd": jax.device_put(Wd_w.astype(np.float16), sh),
                "crt": jax.device_put(np.repeat(
                    np.cbrt(R[..., 0].astype(np.float64)).astype(np.float32),
                    2, axis=0), sh),
                "wxb": jax.device_put(np.tile(np.ascontiguousarray(
                    Wx_b.reshape(RT, 128).T).astype(np.float32), (8, 1)), sh),
                "wdb": jax.device_put(np.tile(np.ascontiguousarray(
                    Wd_b.reshape(RT, 128).T).astype(np.float32), (8, 1)), sh),
            }
            # pipeline: cast chunk j (threads) -> async device_put -> next
            with ThreadPoolExecutor(8) as ex:
                for pref, Vv in (("x", Xv), ("d", Dv)):
                    for j in range(4):
                        chunk = np.empty((n_cores, 512, Din), np.float16)
                        rows = slice(j * 512, (j + 1) * 512)
                        for fut in [ex.submit(chunk.__setitem__, i, Vv[i, rows])
                                    for i in range(n_cores)]:
                            fut.result()
                        globals_map[f"{pref}{j}"] = jax.device_put(
                            chunk.reshape(8 * 512, Din), sh)
            echo = _run_fast(globals_map, zeros=zeros)
            LAST_RESULT = None
        except Exception:
            echo = None

    if echo is None:
        # fallback: the stock run_bass_kernel_spmd path
        with ThreadPoolExecutor(8) as ex:
            futs = [ex.submit(X16.__setitem__, i, Xv[i])
                    for i in range(n_cores)]
            futs += [ex.submit(D16.__setitem__, i, Dv[i])
                     for i in range(n_cores)]
            for fut in futs:
                fut.result()
        wx16 = Wx_w.astype(np.float16)
        wd16 = Wd_w.astype(np.float16)
        crt = np.cbrt(R[..., 0].astype(np.float64)).astype(np.float32)
        wxb = np.ascontiguousarray(Wx_b.reshape(RT, 128).T).astype(np.float32)
        wdb = np.ascontiguousarray(Wd_b.reshape(RT, 128).T).astype(np.float32)
        in_maps = []
        for core in range(n_cores):
            b = core // 2
            m = {
                "wx": wx16[core * 128:(core + 1) * 128],
                "wd": wd16[core * 128:(core + 1) * 128],
                "crt": crt[b][None, :],
                "wxb": wxb,
                "wdb": wdb,
            }
            for j in range(4):
                m[f"x{j}"] = X16[core, j * 512:(j + 1) * 512]
                m[f"d{j}"] = D16[core, j * 512:(j + 1) * 512]
            in_maps.append(m)
        res = run_bass_kernel_spmd(nc, in_maps, core_ids=list(range(n_cores)))
        LAST_RESULT = res
        echo = np.concatenate(
            [res.results[c]["out"] for c in range(n_cores)], 0)

    out = echo.reshape(B, Nx, 1) * np.float32(Wr_w[0, 0]) + np.float32(Wr_b[0])
    return out.astype(np.float32)


_warm()
_build_fast()
_warm_run()


# revision 30
# speedup vs baseline: 1.0920x; 1.0920x over previous
"""Trainium2 Bass kernel for nn_Minerva2 (pooling / cubic-score attention).

Math:
  Xw = X @ Wx_w.T + Wx_b          [B, Nx, Drep]
  Dw = D @ Wd_w.T + Wd_b          [B, Nd, Drep]
  a  = Xw @ Dw.T                  [B, Nx, Nd]
  act = sign(a)*|a|^3 = a^3
  echo = act @ R                  [B, Nx, 1]
  out = echo * Wr_w + Wr_b

Identity: a^3 * R_d = (a * cbrt(R_d))^3, so cbrt(R) is folded into Dw's
columns on-chip (DwT free dim) and the epilogue is a plain cube + row-sum.

The wall-clock of kernel() is dominated by host->device transfer over the
axon tunnel (~60 MB/s), so the design minimizes unique bytes:
  - all large operands are shipped as float16 (PE runs fp16 at full rate,
    fp32 PSUM accumulation keeps rel-err ~6e-4)
  - nothing is duplicated: each core receives only its own shard of X, D,
    and the weights; full D (per batch pair) and full weights are rebuilt
    on-device with AllGather collectives over NeuronLink
  - no host-side transposes: operands ship in native row-major layout and
    are transposed on-chip with the DMA-transpose XBAR (16-bit dtypes)

One-time setup (axon connection, bass build+compile, neuronxcc compile,
NEFF load, collectives init, XLA compile cache) happens at import via a
zero-input warm run, so kernel() itself only pays prep + transfer + exec.

The timed path is fully pipelined: the jitted shard_map is AOT-compiled at
import (zero trace/lower/compile in kernel()); X and D ship as 4 row-chunk
tensors each, so fp16-cast -> async device_put per chunk keeps the tunnel
stream saturated from ~15ms in; the donated output buffer is pre-put ahead
of the bulk; the D2H result copy is enqueued at dispatch. A fallback to the
stock run_bass_kernel_spmd path covers any fast-path failure.

Sharding: core = 2*b + h handles batch b, X-rows half h. 8 cores, no
host-side duplication. Per-core inputs:
  x   [2048, 1024] f16  X[b, h*2048:(h+1)*2048]
  d   [2048, 1024] f16  D[b, h*2048:(h+1)*2048]   (AllGather pair -> D[b])
  wx  [128, 1024]  f16  Wx_w rows [128c:128c+128)  (AllGather all-8 -> Wx_w)
  wd  [128, 1024]  f16  Wd_w rows
  crt [1, 4096]    f32  cbrt(R[b,:,0])
  wxb/wdb [128, 8] f32  biases tiled per r-tile
Output: out [2048, 1] f32 = echo rows (Wr applied on host).
"""

import numpy as np

import concourse.bacc as bacc
import concourse.mybir as mybir
import concourse.tile as tile
from concourse.bass_utils import run_bass_kernel_spmd

F32 = mybir.dt.float32
F32R = mybir.dt.float32r
F16 = mybir.dt.float16
AF = mybir.ActivationFunctionType
ALU = mybir.AluOpType

NXS = 2048   # X rows per core
NDS = 2048   # D rows per core (pre-AllGather shard)
ND = 4096    # full Nd per batch
DIN = 1024
DREP = 1024
KT = DIN // 128    # k-tiles over Din
RT = DREP // 128   # r-tiles over Drep
DC = ND // 512     # Nd chunks of 512
XC = NXS // 512    # Nx chunks of 512
XT = 4             # x-tiles of 128 per x-chunk


def build_nc():
    nc = bacc.Bacc("TRN2")
    # X and D arrive as 4 row-chunks each so the host can pipeline
    # fp16-cast -> device_put per chunk (first bytes on the wire early)
    xs_d = [nc.dram_tensor(f"x{j}", [512, DIN], F16, kind="ExternalInput")
            for j in range(XC)]
    ds_d = [nc.dram_tensor(f"d{j}", [512, DIN], F16, kind="ExternalInput")
            for j in range(NDS // 512)]
    wx_d = nc.dram_tensor("wx", [128, DIN], F16, kind="ExternalInput")
    wd_d = nc.dram_tensor("wd", [128, DIN], F16, kind="ExternalInput")
    crt_d = nc.dram_tensor("crt", [1, ND], F32, kind="ExternalInput")
    wxb_d = nc.dram_tensor("wxb", [128, RT], F32, kind="ExternalInput")
    wdb_d = nc.dram_tensor("wdb", [128, RT], F32, kind="ExternalInput")
    out_d = nc.dram_tensor("out", [NXS, 1], F32, kind="ExternalOutput")

    with tile.TileContext(nc) as tc:
        with (
            tc.tile_pool(name="dram", bufs=1, space="DRAM") as dram,
            tc.tile_pool(name="wt", bufs=1) as wt_pool,
            tc.tile_pool(name="dwt", bufs=1) as dwt_pool,
            tc.tile_pool(name="misc", bufs=1) as misc_pool,
            tc.tile_pool(name="psum", bufs=8, space="PSUM") as psum_pool,
            tc.tile_pool(name="dt_s", bufs=16) as dt_pool,
            tc.tile_pool(name="xt_s", bufs=16) as xt_pool,
            tc.tile_pool(name="xwt", bufs=2) as xwt_pool,
            tc.tile_pool(name="epi", bufs=4) as epi_pool,
        ):
            # ---- collectives: rebuild full D (pair) and weights (all-8) ----
            d_in = dram.tile([NDS, DIN], F16, name="d_in")
            d_all = dram.tile([ND, DIN], F16, name="d_all")
            for j in range(NDS // 512):
                nc.gpsimd.dma_start(d_in[j * 512:(j + 1) * 512, :],
                                    ds_d[j][:, :])
            nc.gpsimd.collective_compute(
                "AllGather", ALU.bypass,
                replica_groups=[[0, 1], [2, 3], [4, 5], [6, 7]],
                ins=[d_in.opt()], outs=[d_all.opt()],
            )
            wx_in = dram.tile([128, DIN], F16, name="wx_in")
            wx_all = dram.tile([DREP, DIN], F16, name="wx_all",
                               addr_space="Shared")
            nc.gpsimd.dma_start(wx_in[:], wx_d[:, :])
            nc.gpsimd.collective_compute(
                "AllGather", ALU.bypass,
                replica_groups=[[0, 1, 2, 3, 4, 5, 6, 7]],
                ins=[wx_in.opt()], outs=[wx_all.opt()],
            )
            wd_in = dram.tile([128, DIN], F16, name="wd_in")
            wd_all = dram.tile([DREP, DIN], F16, name="wd_all",
                               addr_space="Shared")
            nc.gpsimd.dma_start(wd_in[:], wd_d[:, :])
            nc.gpsimd.collective_compute(
                "AllGather", ALU.bypass,
                replica_groups=[[0, 1, 2, 3, 4, 5, 6, 7]],
                ins=[wd_in.opt()], outs=[wd_all.opt()],
            )

            # ---- weights to SBUF, transposed: wxt[k] = WxT[128k:,:] ----
            wxt = []
            wdt = []
            for k in range(KT):
                t = wt_pool.tile([128, DREP], F16, name=f"wxt{k}")
                nc.sync.dma_start_transpose(
                    t[:], wx_all[0:DREP, k * 128:(k + 1) * 128])
                wxt.append(t)
                t = wt_pool.tile([128, DREP], F16, name=f"wdt{k}")
                nc.sync.dma_start_transpose(
                    t[:], wd_all[0:DREP, k * 128:(k + 1) * 128])
                wdt.append(t)

            # ---- biases ----
            wxb = misc_pool.tile([128, RT], F32, name="wxb")
            nc.sync.dma_start(wxb[:], wxb_d[:, :])
            wdb = misc_pool.tile([128, RT], F32, name="wdb")
            nc.sync.dma_start(wdb[:], wdb_d[:, :])

            # ---- crt broadcast tiles: crtb[c][p, f] = cbrt(R[512c+f]) ----
            crt_sb = misc_pool.tile([1, ND], F32, name="crt_sb")
            nc.sync.dma_start(crt_sb[:], crt_d[:, :])
            crtb = []
            for c in range(DC):
                t = misc_pool.tile([128, 512], F32, name=f"crtb{c}")
                nc.gpsimd.partition_broadcast(
                    t[:], crt_sb[:, c * 512:(c + 1) * 512])
                crtb.append(t)

            # ---- Phase D: DwT[r] [128, ND] = (Wd D^T + bd) * crt ----
            dwt = [
                dwt_pool.tile([128, ND], F16, name=f"dwt{r}")
                for r in range(RT)
            ]
            for c in range(DC):
                dts = []
                for k in range(KT):
                    t = dt_pool.tile([128, 512], F16, name=f"dt{c}_{k}",
                                     tag="dt")
                    nc.sync.dma_start_transpose(
                        t[:],
                        d_all[c * 512:(c + 1) * 512, k * 128:(k + 1) * 128])
                    dts.append(t)
                psums = [
                    psum_pool.tile([128, 512], F32, name=f"pd{c}_{r}", tag="ps")
                    for r in range(RT)
                ]
                for k in range(KT):
                    for r in range(RT):
                        nc.tensor.matmul(
                            psums[r][:],
                            wdt[k][:, r * 128:(r + 1) * 128],
                            dts[k][:],
                            start=(k == 0), stop=(k == KT - 1),
                        )
                for r in range(RT):
                    # dwt = (psum + bd[r]) * crt, fused on vector engine
                    nc.vector.scalar_tensor_tensor(
                        out=dwt[r][:, c * 512:(c + 1) * 512],
                        in0=psums[r][:], scalar=wdb[:, r:r + 1],
                        in1=crtb[c][:],
                        op0=ALU.add, op1=ALU.mult,
                    )

            # ---- Phase X + S per x-chunk ----
            for xc in range(XC):
                xts = []
                for k in range(KT):
                    t = xt_pool.tile([128, 512], F16, name=f"xt{xc}_{k}",
                                     tag="xt")
                    nc.sync.dma_start_transpose(
                        t[:], xs_d[xc][0:512, k * 128:(k + 1) * 128])
                    xts.append(t)
                psums = [
                    psum_pool.tile([128, 512], F32, name=f"px{xc}_{r}", tag="ps")
                    for r in range(RT)
                ]
                for k in range(KT):
                    for r in range(RT):
                        nc.tensor.matmul(
                            psums[r][:],
                            wxt[k][:, r * 128:(r + 1) * 128],
                            xts[k][:],
                            start=(k == 0), stop=(k == KT - 1),
                        )
                xwt = [
                    xwt_pool.tile([128, 512], F16, name=f"xwt{xc}_{r}",
                                  tag=f"xwt{r}")
                    for r in range(RT)
                ]
                for r in range(RT):
                    # XwT = psum + bx[r]  (per-partition bias)
                    nc.scalar.activation(xwt[r][:], psums[r][:], AF.Identity,
                                         bias=wxb[:, r:r + 1])

                # --- score + cube + reduce per x-tile ---
                for xi in range(XT):
                    gx = xc * 512 + xi * 128
                    spsum = [
                        psum_pool.tile([128, 512], F32, name=f"s{xc}_{xi}_{d}",
                                       tag="ps")
                        for d in range(DC)
                    ]
                    for r in range(RT):
                        for d in range(DC):
                            nc.tensor.matmul(
                                spsum[d][:],
                                xwt[r][:, xi * 128:(xi + 1) * 128],
                                dwt[r][:, d * 512:(d + 1) * 512],
                                start=(r == 0), stop=(r == RT - 1),
                            )
                    acc = epi_pool.tile([128, DC], F32, name=f"acc{xc}_{xi}",
                                        tag="acc")
                    for d in range(DC):
                        sq = epi_pool.tile([128, 512], F32,
                                           name=f"sq{xc}_{xi}_{d}", tag="sq")
                        nc.scalar.activation(sq[:], spsum[d][:], AF.Square)
                        t3 = epi_pool.tile([128, 512], F32,
                                           name=f"t3{xc}_{xi}_{d}", tag="t3")
                        nc.vector.scalar_tensor_tensor(
                            out=t3[:], in0=sq[:], scalar=1.0, in1=spsum[d][:],
                            op0=ALU.mult, op1=ALU.mult,
                            accum_out=acc[:, d:d + 1],
                        )
                    echo = epi_pool.tile([128, 1], F32, name=f"e{xc}_{xi}",
                                         tag="echo")
                    nc.vector.reduce_sum(echo[:], acc[:],
                                         axis=mybir.AxisListType.X)
                    nc.sync.dma_start(out_d[gx:gx + 128, :], echo[:])

    nc.compile()
    return nc


_NC = None


def _get_nc():
    global _NC
    if _NC is None:
        _NC = build_nc()
    return _NC


def _warm():
    """One-time environment setup: axon device init + connection warmup,
    and the persistent XLA compile cache so repeat runs skip jit compile."""
    try:
        import jax
        jax.config.update("jax_compilation_cache_dir", "/root/.jax_xla_cache")
        jax.config.update("jax_persistent_cache_min_entry_size_bytes", -1)
        jax.config.update("jax_persistent_cache_min_compile_time_secs", 0.0)
        devs = jax.devices()
        z = np.zeros((8, 1), np.float32)
        from jax.sharding import Mesh, PartitionSpec, NamedSharding
        mesh = Mesh(np.asarray(devs), ("core",))
        jax.block_until_ready(
            jax.device_put(z, NamedSharding(mesh, PartitionSpec("core"))))
    except Exception:
        pass


_FAST = None
_GLOBAL_SHAPES = {
    **{f"x{j}": ((8 * 512, DIN), np.float16) for j in range(XC)},
    **{f"d{j}": ((8 * 512, DIN), np.float16) for j in range(NDS // 512)},
    "wx": ((8 * 128, DIN), np.float16),
    "wd": ((8 * 128, DIN), np.float16),
    "crt": ((8, ND), np.float32),
    "wxb": ((8 * 128, RT), np.float32),
    "wdb": ((8 * 128, RT), np.float32),
}


def _build_fast():
    """AOT-compile the jitted shard_map at import so the timed call pays no
    trace/lower/compile — only transfer + execute. Mirrors the axon branch
    of run_bass_kernel_spmd (bass2jax.run_bass_via_pjrt) exactly."""
    global _FAST
    try:
        import jax
        from jax.sharding import Mesh, PartitionSpec, NamedSharding
        import warnings
        with warnings.catch_warnings():
            warnings.simplefilter("ignore")
            from jax.experimental.shard_map import shard_map
        from concourse.bass2jax import (
            _bass_exec_p, partition_id_tensor, install_neuronx_cc_hook)

        nc = _get_nc()
        install_neuronx_cc_hook()
        partition_name = (nc.partition_id_tensor.name
                          if nc.partition_id_tensor else None)
        in_names, out_names, out_avals = [], [], []
        for alloc in nc.m.functions[0].allocations:
            if not isinstance(alloc, mybir.MemoryLocationSet):
                continue
            name = alloc.memorylocations[0].name
            if alloc.kind == "ExternalInput":
                if name != partition_name:
                    in_names.append(name)
            elif alloc.kind == "ExternalOutput":
                out_names.append(name)
                out_avals.append(jax.core.ShapedArray(
                    tuple(alloc.tensor_shape), mybir.dt.np(alloc.dtype)))
        n_params = len(in_names)
        in_names_full = in_names + out_names + (
            [partition_name] if partition_name else [])
        donate = tuple(range(n_params, n_params + len(out_names)))

        def _body(*args):
            operands = list(args)
            if partition_name:
                operands.append(partition_id_tensor())
            return tuple(_bass_exec_p.bind(
                *operands, out_avals=tuple(out_avals),
                in_names=tuple(in_names_full), out_names=tuple(out_names),
                lowering_input_output_aliases=(),
                sim_require_finite=True, sim_require_nnan=True, nc=nc))

        devices = jax.devices()[:8]
        mesh = Mesh(np.asarray(devices), ("core",))
        sh = NamedSharding(mesh, PartitionSpec("core"))
        sharded = jax.jit(
            shard_map(_body, mesh=mesh,
                      in_specs=(PartitionSpec("core"),) * (n_params + 1),
                      out_specs=(PartitionSpec("core"),) * len(out_names),
                      check_rep=False),
            donate_argnums=donate, keep_unused=True)
        sds = [jax.ShapeDtypeStruct(*_GLOBAL_SHAPES[n], sharding=sh)
               for n in in_names]
        sdz = [jax.ShapeDtypeStruct((8 * NXS, 1), np.float32, sharding=sh)]
        compiled = sharded.lower(*sds, *sdz).compile()
        _FAST = {"compiled": compiled, "sh": sh, "in_names": in_names}
    except Exception:
        _FAST = None


def _run_fast(globals_map, zeros=None):
    """Transfer pre-built global arrays and execute the AOT executable.
    Returns echo [8*NXS, 1] f32."""
    import jax
    f = _FAST
    sh = f["sh"]
    if zeros is None:
        zeros = np.zeros((8 * NXS, 1), np.float32)
    dev_in = [jax.device_put(globals_map[n], sh) for n in f["in_names"]]
    outs = f["compiled"](*dev_in, zeros)
    try:
        # enqueue the D2H copy now so the bytes stream the moment the
        # execution completes instead of after a blocking round-trip
        outs[0].copy_to_host_async()
    except Exception:
        pass
    return np.asarray(outs[0])


def _warm_run():
    """Import-time warm run with zero inputs: populates the persistent XLA
    cache, loads the NEFF onto the cores, and initializes the collectives,
    so the first real kernel() call skips all one-time setup."""
    try:
        if _FAST is not None:
            zmap = {n: np.zeros(s, d)
                    for n, (s, d) in _GLOBAL_SHAPES.items()}
            _run_fast(zmap)
        else:
            nc = _get_nc()
            zmap = {
                "wx": np.zeros((128, DIN), np.float16),
                "wd": np.zeros((128, DIN), np.float16),
                "crt": np.zeros((1, ND), np.float32),
                "wxb": np.zeros((128, RT), np.float32),
                "wdb": np.zeros((128, RT), np.float32),
            }
            for j in range(4):
                zmap[f"x{j}"] = np.zeros((512, DIN), np.float16)
                zmap[f"d{j}"] = np.zeros((512, DIN), np.float16)
            run_bass_kernel_spmd(nc, [zmap] * 8, core_ids=list(range(8)))
    except Exception:
        pass


LAST_RESULT = None


def kernel(X, D, R, Wx_w, Wx_b, Wd_w, Wd_b, Wr_w, Wr_b):
    global LAST_RESULT
    n_cores = 8
    X = np.asarray(X)
    D = np.asarray(D)
    R = np.asarray(R)
    Wx_w = np.asarray(Wx_w)
    Wx_b = np.asarray(Wx_b)
    Wd_w = np.asarray(Wd_w)
    Wd_b = np.asarray(Wd_b)
    Wr_w = np.asarray(Wr_w)
    Wr_b = np.asarray(Wr_b)
    B, Nx, Din = X.shape
    Nd = D.shape[1]

    nc = _get_nc()
    # fp16 casts of the two 64MB operands run in parallel threads
    # (numpy releases the GIL for the conversion loops)
    from concurrent.futures import ThreadPoolExecutor
    X16 = np.empty((n_cores, NXS, Din), np.float16)
    D16 = np.empty((n_cores, NDS, Din), np.float16)
    Xv = X.reshape(n_cores, NXS, Din)
    Dv = D.reshape(n_cores, NDS, Din)

    echo = None
    if _FAST is not None:
        try:
            import jax
            f = _FAST
            sh = f["sh"]
            # donated output buffer: enqueue ahead of the 67MB of chunks so
            # it is resident by dispatch time
            zeros = jax.device_put(np.zeros((8 * NXS, 1), np.float32), sh)
            globals_map = {
                # smalls first: cheap to build, get the stream going
                "wx": jax.device_put(Wx_w.astype(np.float16), sh),
                "wd": jax.device_put(Wd_w.astype(np.float16), sh),
                "crt": jax.device_put(np.repeat(
                    np.cbrt(R[..., 0].astype(np.float64)).astype(np.float32),
                    2, axis=0), sh),
                "wxb": jax.device_put(np.tile(np.ascontiguousarray(
                    Wx_b.reshape(RT, 128).T).astype(np.float32), (8, 1)), sh),
                "wdb": jax.device_put(np.tile(np.ascontiguousarray(
                    Wd_b.reshape(RT, 128).T).astype(np.float32), (8, 1)), sh),
            }
            # pipeline: cast chunk j (threads) -> async device_put -> next
            with ThreadPoolExecutor(8) as ex:
                for pref, Vv in (("x", Xv), ("d", Dv)):
                    for j in range(4):
                        chunk = np.empty((n_cores, 512, Din), np.float16)
                        rows = slice(j * 512, (j + 1) * 512)
                        for fut in [ex.submit(chunk.__setitem__, i, Vv[i, rows])
                                    for i in range(n_cores)]:
                            fut.result()
                        globals_map[f"{pref}{j}"] = jax.device_put(
                            chunk.reshape(8 * 512, Din), sh)
            echo = _run_fast(globals_map, zeros=zeros)
            LAST_RESULT = None
        except Exception:
            echo = None

    if echo is None:
        # fallback: the stock run_bass_kernel_spmd path
        with ThreadPoolExecutor(8) as ex:
            futs = [ex.submit(X16.__setitem__, i, Xv[i])
                    for i in range(n_cores)]
            futs += [ex.submit(D16.__setitem__, i, Dv[i])
                     for i in range(n_cores)]
            for fut in futs:
                fut.result()
        wx16 = Wx_w.astype(np.float16)
        wd16 = Wd_w.astype(np.float16)
        crt = np.cbrt(R[..., 0].astype(np.float64)).astype(np.float32)
        wxb = np.ascontiguousarray(Wx_b.reshape(RT, 128).T).astype(np.float32)
        wdb = np.ascontiguousarray(Wd_b.reshape(RT, 128).T).astype(np.float32)
        in_maps = []
        for core in range(n_cores):
            b = core // 2
            m = {
                "wx": wx16[core * 128:(core + 1) * 128],
                "wd": wd16[core * 128:(core + 1) * 128],
                "crt": crt[b][None, :],
                "wxb": wxb,
                "wdb": wdb,
            }
            for j in range(4):
                m[f"x{j}"] = X16[core, j * 512:(j + 1) * 512]
                m[f"d{j}"] = D16[core, j * 512:(j + 1) * 512]
            in_maps.append(m)
        res = run_bass_kernel_spmd(nc, in_maps, core_ids=list(range(n_cores)))
        LAST_RESULT = res
        echo = np.concatenate(
            [res.results[c]["out"] for c in range(n_cores)], 0)

    out = echo.reshape(B, Nx, 1) * np.float32(Wr_w[0, 0]) + np.float32(Wr_b[0])
    return out.astype(np.float32)


_warm()
_build_fast()
_warm_run()
